# revision 2
# baseline (speedup 1.0000x reference)
"""Trainium2 Bass kernel for nn_ModelWithSTMGNNLayer (GAT-style message passing
+ global-memory cross-attention), distributed over 8 NeuronCores.

Sharding: nodes split into 8 contiguous shards (graph parallel); edges
partitioned by destination-node owner. Per layer: each core computes its
shard's h/e_src/e_dst with one fused matmul, shards are bf16-AllGathered,
edges are processed with dma_gather + one-hot scatter matmuls, and the
node update (memory attention + residual) is accumulated in PSUM.
"""
import sys
for _p in ("/opt/trn_rl_repo",):
    if _p not in sys.path:
        sys.path.insert(0, _p)

import numpy as np

import concourse.bacc as bacc
import concourse.mybir as mybir
import concourse.tile as tile
from concourse.bass_utils import run_bass_kernel_spmd
from concourse.library_config import mlp

# problem constants
N, E, FEAT, C, H, D, L, T, MS, MD, NCOUT = 20000, 320000, 128, 256, 8, 32, 5, 64, 10, 128, 2
NEG = 0.2
W = 8                  # cores
NSH = N // W           # 2500 nodes per core
P = 128
NB = (NSH + P - 1) // P          # 20 dst blocks per core
NLOC = NB * P                    # 2560 padded local nodes
ROW = 384                        # bf16 h_ext row: h(256) | es_hi(8) | es_lo(8) | pad
EDR = 64                         # fp32 e_dst row
NCH = 5                          # 512-wide node chunks for x0 matmul

f32 = mybir.dt.float32
f32r = mybir.dt.float32r
bf16 = mybir.dt.bfloat16
i16 = mybir.dt.int16

# feature permutation: new col j = d*H + h  <=>  old col = h*D + d
PERM = np.array([(j % H) * D + (j // H) for j in range(C)], dtype=np.int64)


# ----------------------------------------------------------------- host prep

def _wrap_idx(v, cap):
    """dma_gather index layout: idx i at [i % 16, i // 16], tiled to 128 rows."""
    a = np.zeros((16, cap // 16), np.int16)
    a[np.arange(len(v)) % 16, np.arange(len(v)) // 16] = v.astype(np.int16)
    return np.tile(a, (8, 1))


def preprocess_graph(edge_index):
    """Partition edges by dst owner into per-(core, block) lists, padded to a
    uniform cap. Returns cap_tiles and per-core static index arrays."""
    src = np.asarray(edge_index[0], dtype=np.int64)
    dst = np.asarray(edge_index[1], dtype=np.int64)
    owner = dst // NSH
    loc = dst % NSH
    blk = loc // P
    dloc = loc % P
    # gather row id in the AllGather layout (2560-row shards)
    srow = (src // NSH) * NLOC + (src % NSH)

    counts = np.zeros((W, NB), np.int64)
    np.add.at(counts, (owner, blk), 1)
    cap = int(np.ceil(counts.max() / P) * P)
    ct = cap // P

    order = np.lexsort((dloc, blk, owner))
    so, sb = owner[order], blk[order]
    ssrow, sdloc, sloc = srow[order], dloc[order], loc[order]
    starts = np.zeros((W, NB), np.int64)
    flat = (so * NB + sb)
    # start offset of each (core, block) group in the sorted arrays
    first = np.searchsorted(flat, np.arange(W * NB), side="left")

    per_core = []
    for c in range(W):
        isrc = np.zeros((NB, P, cap // 16), np.int16)
        idst = np.zeros((NB, P, cap // 16), np.int16)
        dl = np.full((NB, ct * P), -1.0, np.float32)
        for b in range(NB):
            g = c * NB + b
            n = int(counts[c, b])
            s0 = int(first[g])
            rows = np.zeros(cap, np.int64)
            dsts = np.zeros(cap, np.int64)
            rows[:n] = ssrow[s0:s0 + n]
            dsts[:n] = sloc[s0:s0 + n]
            dl[b, :n] = sdloc[s0:s0 + n]
            isrc[b] = _wrap_idx(rows, cap)
            idst[b] = _wrap_idx(dsts, cap)
        # dloc layout [P, ct]: edge t*128+p at [p, t]
        dlw = dl.reshape(NB, ct, P).transpose(0, 2, 1)
        per_core.append((isrc, idst, dlw.astype(np.float32)))
    return cap, per_core


def preprocess_weights(inp):
    """Fold time-proj, attention vectors, memory K/Q/V into per-layer consts
    (all in the d-major permuted feature space)."""
    Wg, Wt = np.asarray(inp["Wg"]), np.asarray(inp["Wt"])
    a_src, a_dst = np.asarray(inp["a_src"]), np.asarray(inp["a_dst"])
    Wq, Wk, Wv = np.asarray(inp["Wq"]), np.asarray(inp["Wk"]), np.asarray(inp["Wv"])
    mem = np.asarray(inp["global_memory"])

    # A_src[c=h*D+d, h2] = a[h2, d] * (h == h2)
    def a_mat(a_l):
        A = np.zeros((C, H), np.float32)
        for h in range(H):
            A[h * D:(h + 1) * D, h] = a_l[h]
        return A

    rhs_all, wqk_all, v_all = [], [], []
    for l in range(L):
        R_es = Wg[l] @ a_mat(a_src[l])            # [C, H]
        R_ed = Wg[l] @ a_mat(a_dst[l])
        Wg_p = Wg[l][np.ix_(PERM, PERM)]          # rows+cols permuted
        Rx = np.concatenate([Wg_p, R_es[PERM], R_ed[PERM]], axis=1)   # [256, 272]
        Wt_p = Wt[l][:, PERM]
        Rte = np.concatenate([Wt[l] @ Wg[l][:, PERM], Wt[l] @ R_es, Wt[l] @ R_ed],
                             axis=1)              # [64, 272]
        rhs = np.zeros((3, P, C + 2 * H), np.float32)
        rhs[0] = Rx[:P]
        rhs[1] = Rx[P:]
        rhs[2, :T] = Rte
        rhs_all.append(rhs)

        k = mem @ Wk[l]                            # [MS, C]
        wqk = (Wq[l] @ k.T) / np.sqrt(C)           # [C, MS]
        wqk_all.append(wqk[PERM].astype(np.float32))
        v = mem @ Wv[l]                            # [MS, C]
        v_all.append(v[:, PERM].astype(np.float32))
    return (np.stack(rhs_all).astype(np.float32),          # [L, 3, 128, 272]
            np.stack(wqk_all),                             # [L, 256, 10]
            np.stack(v_all))                               # [L, 10, 256]


# ------------------------------------------------------------------ builder

def build_nc(ct, l_run=L, dbg=False, dbg_stages=False, use_ag=True, use_gather=True):
    """ct = cap // 128 edge tiles per block."""
    cap = ct * P
    nc = bacc.Bacc("TRN2", num_devices=W)

    xiT = nc.dram_tensor("xiT", [FEAT, NLOC], f32r, kind="ExternalInput")
    teT = nc.dram_tensor("teT", [T, NLOC], f32r, kind="ExternalInput")
    wi = nc.dram_tensor("wi", [FEAT, C], f32r, kind="ExternalInput")
    rhs_d = nc.dram_tensor("rhs", [l_run, 3, P, C + 2 * H], f32r, kind="ExternalInput")
    wqk_d = nc.dram_tensor("wqk", [l_run, C, MS], f32r, kind="ExternalInput")
    v_d = nc.dram_tensor("v", [l_run, MS, C], f32r, kind="ExternalInput")
    ident_d = nc.dram_tensor("ident", [P, P], f32r, kind="ExternalInput")
    iota_d = nc.dram_tensor("iota", [P, P], bf16, kind="ExternalInput")
    isrc_d = nc.dram_tensor("isrc", [NB, P, cap // 16], i16, kind="ExternalInput")
    idst_d = nc.dram_tensor("idst", [NB, P, cap // 16], i16, kind="ExternalInput")
    dloc_d = nc.dram_tensor("dloc", [NB, P, ct], f32, kind="ExternalInput")

    pooled_d = nc.dram_tensor("pooled", [P, 2], f32, kind="ExternalOutput")
    if dbg:
        xdbg = nc.dram_tensor("xdbg", [l_run, C, NLOC], f32, kind="ExternalOutput")
    if dbg_stages:
        hxdbg = nc.dram_tensor("hxdbg", [l_run, P, NB, ROW], bf16, kind="ExternalOutput")
        nndbg = nc.dram_tensor("nndbg", [l_run, P, NB, C], f32, kind="ExternalOutput")
        atdbg = nc.dram_tensor("atdbg", [l_run, P, NB, MS], f32, kind="ExternalOutput")

    AluOp, ActF, AxL = mybir.AluOpType, mybir.ActivationFunctionType, mybir.AxisListType

    with tile.TileContext(nc) as tc:
        with (
            tc.tile_pool(name="const", bufs=1) as cst,
            tc.tile_pool(name="xt", bufs=2) as xtp,
            tc.tile_pool(name="stage_a", bufs=2) as sta,
            tc.tile_pool(name="hx", bufs=1) as hxp,
            tc.tile_pool(name="gather", bufs=2) as gat,
            tc.tile_pool(name="edge", bufs=2) as edg,
            tc.tile_pool(name="node", bufs=3) as nod,
            tc.tile_pool(name="small", bufs=3) as sml,
            tc.tile_pool(name="psA", bufs=2, space="PSUM") as psA,
            tc.tile_pool(name="psB", bufs=2, space="PSUM") as psB,
            tc.tile_pool(name="psC", bufs=1, space="PSUM") as psC,
            tc.tile_pool(name="dram", bufs=1, space="DRAM") as drm,
        ):
            nc.gpsimd.load_library(mlp)

            # ---------- constants to SBUF
            ident = cst.tile([P, P], f32r)
            nc.sync.dma_start(ident[:], ident_d[:])
            iota = cst.tile([P, P], bf16)
            nc.sync.dma_start(iota[:], iota_d[:])
            wqk_sb = cst.tile([P, l_run * 2, MS], f32r)
            nc.sync.dma_start(wqk_sb[:], wqk_d[:].rearrange("l (k p) m -> p (l k) m", p=P))
            v_sb = cst.tile([MS, l_run, C], f32r)
            nc.sync.dma_start(v_sb[:], v_d[:].rearrange("l m f -> m l f"))
            teT_sb = cst.tile([T, NLOC], f32r)
            nc.sync.dma_start(teT_sb[:], teT[:])
            wi_sb = cst.tile([P, 2, P], f32r)
            nc.sync.dma_start(wi_sb[:], wi[:].rearrange("f (k p) -> f k p", k=2))

            # ---------- DRAM internal buffers
            ag_in = drm.tile([NLOC, ROW], bf16)
            a_ed = drm.tile([NLOC, EDR], f32)

            # ---------- x0T = relu(Wi'.T @ xiT)  (feature-major, f32r)
            xT = xtp.tile([P, 2, NLOC], f32r, tag="xT")
            for j in range(NCH):
                xi_sb = sml.tile([FEAT, 512], f32r, tag="xi")
                nc.sync.dma_start(xi_sb[:], xiT[:, j * 512:(j + 1) * 512])
                for k in range(2):
                    x0_ps = psA.tile([P, 512], f32, space="PSUM", tag="A")
                    nc.tensor.matmul(x0_ps[:], wi_sb[:, k, :], xi_sb[:],
                                     start=True, stop=True)
                    nc.scalar.activation(xT[:, k, j * 512:(j + 1) * 512], x0_ps[:],
                                         ActF.Relu)

            # ---------------- layers
            for l in range(l_run):
                # ===== stage A: fused h | es | ed matmul per node tile
                hx_full = drm.tile([W * NLOC, ROW], bf16, addr_space="Shared",
                                   tag=f"hxf{l}", name=f"hx_full{l}")
                rhs_sb = sta.tile([P, 3, C + 2 * H], f32r, tag="rhs")
                nc.sync.dma_start(rhs_sb[:], rhs_d[l].rearrange("k p f -> p k f"))
                hx_sb = hxp.tile([P, NB, ROW], bf16, tag="hx")
                ed_sb = hxp.tile([P, NB, H], f32, tag="ed")
                for t in range(NB):
                    h_ps = psA.tile([P, 512], f32, space="PSUM", tag="A", name="h_ps")[:, 0:C + 2 * H]
                    sl = slice(t * P, (t + 1) * P)
                    nc.tensor.matmul(h_ps[:], xT[:, 0, sl], rhs_sb[:, 0, :],
                                     start=True, stop=False)
                    nc.tensor.matmul(h_ps[:], xT[:, 1, sl], rhs_sb[:, 1, :],
                                     start=False, stop=False)
                    nc.tensor.matmul(h_ps[:], teT_sb[:, sl], rhs_sb[:T, 2, :],
                                     start=False, stop=True)
                    # h payload (bf16) + es hi/lo + ed
                    nc.vector.tensor_copy(hx_sb[:, t, 0:C], h_ps[:, 0:C])
                    nc.vector.tensor_copy(hx_sb[:, t, C:C + H], h_ps[:, C:C + H])
                    nc.vector.tensor_tensor(out=hx_sb[:, t, C + H:C + 2 * H],
                                            in0=h_ps[:, C:C + H],
                                            in1=hx_sb[:, t, C:C + H],
                                            op=AluOp.subtract)
                    nc.vector.tensor_copy(ed_sb[:, t, :], h_ps[:, C + H:C + 2 * H])

                nc.sync.dma_start(ag_in[:].rearrange("(t p) f -> p t f", p=P), hx_sb[:])
                if dbg_stages:
                    nc.sync.dma_start(hxdbg[l], hx_sb[:])
                nc.sync.dma_start(
                    a_ed[:].rearrange("(t p) f -> p t f", p=P)[:, :, 0:H], ed_sb[:])
                if use_ag:
                    nc.gpsimd.collective_compute(
                        "AllGather", AluOp.bypass,
                        replica_groups=[list(range(W))],
                        ins=[ag_in[:]], outs=[hx_full[:]])
                else:
                    nc.sync.dma_start(hx_full[0:NLOC], ag_in[:])

                # ===== edge stage + node stage per dst block
                xT_new = xtp.tile([P, 2, NLOC], f32r, tag="xT")
                for b in range(NB):
                    ist = sml.tile([P, cap // 16], i16, tag="ist")
                    idt = sml.tile([P, cap // 16], i16, tag="idt")
                    dlc = sml.tile([P, ct], f32, tag="dlc")
                    nc.sync.dma_start(ist[:], isrc_d[b])
                    nc.sync.dma_start(idt[:], idst_d[b])
                    nc.sync.dma_start(dlc[:], dloc_d[b])

                    G = gat.tile([P, ct, ROW], bf16, tag="G")
                    ED = gat.tile([P, ct, EDR], f32, tag="ED")
                    if use_gather:
                        nc.gpsimd.dma_gather(G[:], hx_full[:], ist[:], cap, cap, ROW,
                                             single_packet=False)
                        nc.gpsimd.dma_gather(ED[:], a_ed[:], idt[:], cap, cap, EDR,
                                             single_packet=False)
                    else:
                        nc.vector.memset(G[:], 1.0)
                        nc.vector.memset(ED[:], 1.0)

                    # logits = es_hi + es_lo + ed ; leaky-relu; exp (bf16 out)
                    LG = edg.tile([P, ct, H], f32, tag="LG")
                    nc.vector.tensor_tensor(out=LG[:], in0=G[:, :, C:C + H],
                                            in1=G[:, :, C + H:C + 2 * H], op=AluOp.add)
                    nc.vector.tensor_tensor(out=LG[:], in0=LG[:], in1=ED[:, :, 0:H],
                                            op=AluOp.add)
                    LG2 = edg.tile([P, ct, H], f32, tag="LG2")
                    nc.vector.tensor_scalar(out=LG2[:], in0=LG[:], scalar1=NEG,
                                            scalar2=None, op0=AluOp.mult)
                    nc.vector.tensor_tensor(out=LG[:], in0=LG[:], in1=LG2[:],
                                            op=AluOp.max)
                    EX = edg.tile([P, ct, H], bf16, tag="EX")
                    nc.scalar.activation(EX[:], LG[:], ActF.Exp)

                    # one-hot S  (bf16, 2x mode)
                    S = edg.tile([P, ct, P], bf16, tag="S")
                    dlcb = edg.tile([P, ct], bf16, tag="dlcb")
                    nc.vector.tensor_copy(dlcb[:], dlc[:])
                    nc.vector.tensor_tensor(
                        out=S[:],
                        in0=iota[:].rearrange("p (o n) -> p o n", o=1).broadcast_to([P, ct, P]),
                        in1=dlcb[:].rearrange("p (n o) -> p n o", o=1).broadcast_to([P, ct, P]),
                        op=AluOp.is_equal)

                    # WF = [ ex * h  |  ex ]   (d-major: in1 step-0 middle keeps 2x)
                    WF = edg.tile([P, ct, C + H], bf16, tag="WF")
                    nc.vector.tensor_tensor(
                        out=WF[:, :, 0:C].rearrange("p n (d h) -> p n d h", h=H),
                        in0=G[:, :, 0:C].rearrange("p n (d h) -> p n d h", h=H),
                        in1=EX[:].rearrange("p n (o h) -> p n o h", o=1)
                              .broadcast_to([P, ct, D, H]),
                        op=AluOp.mult)
                    nc.vector.tensor_copy(WF[:, :, C:C + H], EX[:])

                    # scatter: msg[d, :] = sum_e S[e, d] * WF[e, :]
                    msg_ps = psB.tile([P, C + H], f32, space="PSUM", tag="msg")
                    for t in range(ct):
                        nc.tensor.matmul(msg_ps[:], S[:, t, :], WF[:, t, :],
                                         start=(t == 0), stop=(t == ct - 1))

                    # normalize + relu -> node_new (f32r)
                    recip = sml.tile([P, H], f32, tag="recip")
                    nc.vector.tensor_scalar(out=recip[:], in0=msg_ps[:, C:C + H],
                                            scalar1=1e-12, scalar2=None, op0=AluOp.add)
                    nc.vector.reciprocal(recip[:], recip[:])
                    node_new = nod.tile([P, C], f32, tag="nn")
                    nc.vector.tensor_tensor(
                        out=node_new[:].rearrange("p (d h) -> p d h", h=H),
                        in0=msg_ps[:, 0:C].rearrange("p (d h) -> p d h", h=H),
                        in1=recip[:].rearrange("p (o h) -> p o h", o=1)
                              .broadcast_to([P, D, H]),
                        op=AluOp.mult)
                    nc.vector.tensor_scalar(out=node_new[:], in0=node_new[:],
                                            scalar1=0.0, scalar2=None, op0=AluOp.max)

                    if dbg_stages:
                        nc.sync.dma_start(nndbg[l][:, b, :], node_new[:])
                    # ---- node stage for this tile
                    # outT_psum (2 chunks) accumulates nnT + attnvT + xT
                    o_all = psC.tile([P, 2, 512], f32, space="PSUM", tag="ops")
                    o_ps = [o_all[:, k, 0:P] for k in range(2)]
                    nnT = [sml.tile([P, P], f32r, tag=f"nnT{k}", name=f"nnT{k}")
                           for k in range(2)]
                    for k in range(2):
                        nc.tensor.transpose(out=o_ps[k],
                                            in_=node_new[:, k * P:(k + 1) * P],
                                            identity=ident[:].bitcast(f32))
                        nc.vector.tensor_copy(nnT[k][:], o_ps[k])
                    # scores = node' @ Wqk'  -> [128 nodes, MS]
                    sc_ps = psB.tile([P, MS], f32, space="PSUM", tag="scat", bufs=1)
                    nc.tensor.matmul(sc_ps[:], nnT[0][:], wqk_sb[:, l * 2 + 0, :],
                                     start=True, stop=False)
                    nc.tensor.matmul(sc_ps[:], nnT[1][:], wqk_sb[:, l * 2 + 1, :],
                                     start=False, stop=True)
                    # softmax over MS (no max-sub; scores bounded)
                    attn = sml.tile([P, MS], f32, tag="attn")
                    dnm = sml.tile([P, 1], f32, tag="dnm")
                    nc.scalar.activation(attn[:], sc_ps[:], ActF.Exp, accum_out=dnm[:])
                    nc.vector.reciprocal(dnm[:], dnm[:])
                    nc.vector.tensor_scalar(out=attn[:], in0=attn[:], scalar1=dnm[:, 0:1],
                                            scalar2=None, op0=AluOp.mult)
                    if dbg_stages:
                        nc.sync.dma_start(atdbg[l][:, b, :], attn[:])
                    # attnT [MS, 128]
                    at_ps = psB.tile([MS, P], f32, space="PSUM", tag="scat", name="at_ps", bufs=1)
                    nc.tensor.transpose(out=at_ps[:], in_=attn[:], identity=ident[:].bitcast(f32))
                    attnT = sml.tile([MS, P], f32r, tag="attnT")
                    nc.vector.tensor_copy(attnT[:], at_ps[:])
                    # accumulate attnvT + xT into outT psum; relu -> xT_new
                    sl = slice(b * P, (b + 1) * P)
                    for k in range(2):
                        nc.tensor.matmul(o_ps[k], v_sb[:, l, k * P:(k + 1) * P],
                                         attnT[:], start=False, stop=False)
                        nc.tensor.matmul(o_ps[k], ident[:], xT[:, k, sl],
                                         start=False, stop=True)
                        nc.scalar.activation(xT_new[:, k, sl], o_ps[k], ActF.Relu)
                xT = xT_new
                if dbg:
                    xdb = hxp.tile([P, 2, NLOC], f32, tag="xdb")
                    nc.vector.tensor_copy(xdb[:], xT[:])
                    nc.sync.dma_start(xdbg[l].rearrange("(k p) n -> p k n", k=2), xdb[:])

            # ---------- pooled partial (exclude pad nodes)
            pooled = sml.tile([P, 2], f32, tag="pooled")
            nc.vector.tensor_reduce(out=pooled[:], in_=xT[:, :, 0:NSH],
                                    axis=AxL.X, op=AluOp.add)
            nc.sync.dma_start(pooled_d[:], pooled[:])

    nc.compile()
    return nc


# ------------------------------------------------------------------ driver

_CACHED = {}
LAST_EXEC_NS = None


def kernel(**inputs):
    x0 = np.asarray(inputs["x_initial_nodes"], np.float32)
    te = np.asarray(inputs["time_embedding"], np.float32)
    wi = np.asarray(inputs["Wi"], np.float32)[:, PERM].copy()
    bi = np.asarray(inputs["bi"], np.float32)
    assert np.abs(bi).max() == 0.0, "kernel assumes bi == 0"
    rhs, wqk, v = preprocess_weights(inputs)
    cap, per_core = preprocess_graph(np.asarray(inputs["edge_index"]))
    ct = cap // P

    ident = np.eye(P, dtype=np.float32)
    iota = np.tile(np.arange(P, dtype=np.float32), (P, 1))

    import ml_dtypes
    iota_bf = iota.astype(ml_dtypes.bfloat16)
    in_maps = []
    for c in range(W):
        xiT = np.zeros((FEAT, NLOC), np.float32)
        xiT[:, :NSH] = x0[c * NSH:(c + 1) * NSH].T
        teT = np.zeros((T, NLOC), np.float32)
        teT[:, :NSH] = te[c * NSH:(c + 1) * NSH].T
        isrc, idst, dloc = per_core[c]
        in_maps.append({
            "xiT": xiT, "teT": teT, "wi": wi,
            "rhs": rhs, "wqk": wqk, "v": v,
            "ident": ident, "iota": iota_bf,
            "isrc": isrc, "idst": idst, "dloc": dloc,
        })

    key = (ct,)
    if key not in _CACHED:
        _CACHED[key] = build_nc(ct)
    nc = _CACHED[key]
    import os
    trace = os.environ.get("KERNEL_TRACE", "0") == "1"
    tdir = os.environ.get("KERNEL_TRACE_DIR") or None
    res = run_bass_kernel_spmd(nc, in_maps, core_ids=list(range(W)), trace=trace,
                               tmpdir=tdir)
    global LAST_EXEC_NS
    LAST_EXEC_NS = res.exec_time_ns

    # feature j_new = k*128 + p
    pooled_new = np.zeros(C, np.float64)
    for c in range(W):
        po = res.results[c]["pooled"].astype(np.float64)
        pooled_new[0:P] += po[:, 0]
        pooled_new[P:C] += po[:, 1]
    pooled_old = np.empty(C, np.float64)
    pooled_old[PERM] = pooled_new
    pooled_old /= N

    mem = np.asarray(inputs["global_memory"], np.float32)
    mem_pooled = mem.mean(axis=0)
    final = np.concatenate([pooled_old.astype(np.float32), mem_pooled])
    out = final @ np.asarray(inputs["Wc"], np.float32) + np.asarray(inputs["bc"], np.float32)
    return out.astype(np.float32)


if __name__ == "__main__":
    import reference
    inp = {k: np.asarray(v) for k, v in reference.setup_inputs().items()}
    got = kernel(**inp)
    exp = np.asarray(reference.reference(**reference.setup_inputs()))
    err = np.abs(got - exp).max() / (np.abs(exp).max() + 1e-12)
    print("kernel:", got, "\nref:   ", exp, "\nrel err:", err)

